# revision 26
# baseline (speedup 1.0000x reference)
"""Trainium2 Bass kernel for CausalAttentionSortNet bucket-scoring.

Math (see reference): only `k` feeds the output. For each merged batch*head
slice, the cumulative-average of k is sampled at bucket starts (every 128th
row), which reduces to per-chunk sums + a strictly-triangular prefix matmul.
The rest is tiny per-bucket sort projections and a 64x65 masked softmax.

Sharding: data-parallel over the merged (batch*heads)=32 axis across 8 cores,
4 slices per core, processed as 2 pairs of 2 slices; a pair fills the
128-partition dim as partition=(slice_in_pair, chunk), free=(row, dim) so
every partition's k data is contiguous 32KB HBM runs (the single-queue bulk
stream saturates all 16 DMA engines at ~350 GB/s).

`q` (half of all input bytes) is never read by the reference computation, so
it is not even transferred to the device.

DMA-instruction budget: the hardware exposes ~12 DMA completion semaphores;
an instruction >=12 positions later reuses an earlier one's semaphore and
its issue blocks until that user completes, so the constants ship in two
early-completing DMAs and every bulk tile is uniform (a version with a slow
small-packet constant DMA in the reuse chain stalled the bulk queue 12us).
Chunk first-rows are not a separate DMA: they arrive inside each pair's
first bulk tile, whose in-place fold targets the tile's upper half so row 0
survives for the F-term matmuls.

Per-chunk reduction: each pair's rows stream as sub-tiles of
(16x7, 8, 4, 4) rows. Mid-stream, SBUF port contention caps DVE at roughly
1.7ns/elem and GpSimd at ~2.9ns/elem (vs 1.04/2.0 idle) and LARGER chains
degrade further (superlinear contention), so each sub-tile gets an
INDEPENDENT halving-fold chain (contiguous tensor_adds down to one row ->
its own partial-sum slot) and the chains are statically balanced across
both engines: GpSimd takes pair 1's first six chains, DVE everything else
including every chain near the tail. The PE (otherwise idle) folds every
partial into the scaled-prefix via one matmul per sub-tile against the
tril*scale constant, accumulating in that pair's PSUM bank, opened by the
F*diag(s) seed and closed by the last sub-tile's matmul. Small sub-tiles
stream last so the post-stream tail is two short fold chains plus the
epilogue (projections -> 64x65 masked softmax, batched over both pairs).
"""

from contextlib import ExitStack

import numpy as np

import concourse.bacc as bacc
import concourse.mybir as mybir
import concourse.tile as tile
from concourse import bass_utils

# Problem constants (hardcoded per contract; kernel.py must be self-contained).
B, HEADS, BUCKETS, DIM, DIM_SORT, T = 4, 8, 64, 64, 8, 8192
BH = B * HEADS            # 32 merged batch*head slices
NCORES = 8
BHC = BH // NCORES        # 4 slices per core
NPAIR = BHC // 2          # 2 pairs per core
CHUNK = T // BUCKETS      # 128 rows per bucket
NEG = -1.0e30             # softmax mask value (underflows exp to exactly 0)
FP = mybir.dt.float32

# packed-constant column offsets: cpack = [lmat_s | s2col], caux packs the
# nonzero rows of the cq/ck seed blocks plus their row-scatter matrix; the
# other structural constants (identity, diag(s), masks) are built on-chip
# by GpSimd affine_selects during its pre-stream idle window.
NC128 = 128 + 1
NC64 = 4 * 104
NCAUX = 2 * 128 + 104
NCALL = NC128

# pair-1 fold chains for sub-tiles [0, GP_CHAINS) run on GpSimd; all other
# chains (including every chain near the tail) on the faster DVE
GP_CHAINS = 7

TRACE = False  # set by test.py for profiling runs
TRACE_KWARGS = {}  # extra run_bass_kernel_spmd kwargs for profiling runs
LAST_RESULTS = None  # BassKernelResults of the most recent run

_PROG_CACHE = {}


def _cascade_sizes(chunk):
    # uniform mid-size tiles, small ones last: (16x7, 8, 4, 4) for chunk=128
    assert chunk == 128, "sub-tile schedule is tuned for chunk=128"
    sizes = [16] * 7 + [8, 4, 4]
    assert sum(sizes) == chunk, (sizes, chunk)
    return sizes


def _build_program(t_seq=T, enable_asserts=False, debug_taps=False):
    chunk = t_seq // BUCKETS
    sizes = _cascade_sizes(chunk)
    nsub = len(sizes)

    nc = bacc.Bacc(
        "TRN2",
        target_bir_lowering=False,
        debug=False,
        enable_asserts=enable_asserts,
        num_devices=NCORES,
    )

    def din(name, shape):
        return nc.dram_tensor(name, shape, FP, kind="ExternalInput").ap()

    kin = din("kin", (BHC, t_seq, DIM))
    # packed constants, three small DMAs:
    # cpack (128, 129)  [lmat_s | s-column]
    # c64   (64, 416)   [wqk_pt_p0 | wqk_pt_p1 | wqk_ft_p0 | wqk_ft_p1]
    # caux  (32, 360)   [c104 packed nonzero rows (256) | row-scatter P (104)]
    cpack = din("cpack", (128, NCALL))
    c64 = din("c64", (64, NC64))
    caux = din("caux", (32, NCAUX))
    rout = nc.dram_tensor(
        "rout", (BHC, BUCKETS, BUCKETS + 1), FP, kind="ExternalOutput"
    ).ap()

    X = mybir.AxisListType.X
    Exp = mybir.ActivationFunctionType.Exp
    MULT = mybir.AluOpType.mult

    with tile.TileContext(nc) as tc:
        with ExitStack() as ctx:
            singles = ctx.enter_context(tc.tile_pool(name="singles", bufs=1))
            kpools = [
                ctx.enter_context(tc.tile_pool(name=f"kpool{s}", bufs=2))
                for s in range(nsub)
            ]
            parp = ctx.enter_context(tc.tile_pool(name="parp", bufs=nsub))
            small = ctx.enter_context(tc.tile_pool(name="small", bufs=2))
            pp = ctx.enter_context(tc.tile_pool(name="pp", bufs=1, space="PSUM"))

            cp_sb = singles.tile([128, NCALL], FP, tag="cpack")
            nc.scalar.dma_start(cp_sb[:], cpack)
            c64_sb = singles.tile([64, NC64], FP, tag="c64")
            nc.scalar.dma_start(c64_sb[:], c64)
            caux_sb = singles.tile([32, NCAUX], FP, tag="caux")
            nc.scalar.dma_start(caux_sb[:], caux)

            # ---- bulk k sub-tile DMAs, single queue, pair 1 leading so
            # its GpSimd chains start first (contiguous rows*256B runs per
            # partition)
            ksrcs = [
                kin[2 * p : 2 * p + 2].rearrange("b (c r) d -> (b c) r d", r=chunk)
                for p in range(NPAIR)
            ]
            kts = {}
            r0 = 0
            for s, rs in enumerate(sizes):
                for p in (1, 0):
                    kt = kpools[s].tile(
                        [128, rs, DIM], FP, tag=f"kt{s}", name=f"kt{s}_{p}"
                    )
                    nc.sync.dma_start(kt[:], ksrcs[p][:, r0 : r0 + rs, :])
                    kts[(p, s)] = kt
                r0 += rs

            lmat_s = cp_sb[:, 0:128]
            s2col = cp_sb[:, 128:129]

            # on-chip structural constants, built by GpSimd (idle until the
            # first bulk tile lands ~11us in) + one Scalar scale op:
            #   eye      128x128 identity
            #   idents   diag(s) (cumavg scales)
            #   am68/mm68: additive causal mask / tril(-1) output mask in the
            #   68-wide-per-pair logit layout (col 0 pad, col 1+j logit j
            #   valid iff j <= row, cols 66:68 pad)
            # neuronxcc implements only is_ge / is_gt / not_equal for
            # affine_select, so every predicate is phrased with those
            GE, GT, NE = (
                mybir.AluOpType.is_ge,
                mybir.AluOpType.is_gt,
                mybir.AluOpType.not_equal,
            )
            eye_sb = singles.tile([128, 128], FP, tag="eye")
            nc.gpsimd.memset(eye_sb[:], 0.0)
            # where(c - i != 0, 0, fill=1) = identity
            nc.gpsimd.affine_select(
                eye_sb[:], eye_sb[:], [[1, 128]], NE, 1.0,
                base=0, channel_multiplier=-1,
            )
            idents_sb = singles.tile([128, 128], FP, tag="idents")
            nc.scalar.activation(
                idents_sb[:], eye_sb[:], mybir.ActivationFunctionType.Copy,
                scale=s2col,
            )
            am_sb = singles.tile([128, 2, 68], FP, tag="am68")
            nc.gpsimd.memset(am_sb[:], 0.0)
            for b in range(2):
                # keep 0 where row - pos + 1 >= 0 (valid logit), else NEG
                nc.gpsimd.affine_select(
                    am_sb[64 * b : 64 * b + 64], am_sb[64 * b : 64 * b + 64],
                    [[0, 2], [-1, 68]], GE, NEG,
                    base=1, channel_multiplier=1,
                )
            # pad columns: keep pos - 1 >= 0, keep 65 - pos >= 0, else NEG
            nc.gpsimd.affine_select(
                am_sb[:], am_sb[:], [[0, 2], [1, 68]], GE, NEG, base=-1,
                channel_multiplier=0,
            )
            nc.gpsimd.affine_select(
                am_sb[:], am_sb[:], [[0, 2], [-1, 68]], GE, NEG, base=65,
                channel_multiplier=0,
            )
            mm_sb = singles.tile([128, 2, 68], FP, tag="mm68")
            nc.gpsimd.memset(mm_sb[:], 1.0)
            for b in range(2):
                # keep 1 where row - pos + 1 > 0 (output tril(-1)), else 0
                nc.gpsimd.affine_select(
                    mm_sb[64 * b : 64 * b + 64], mm_sb[64 * b : 64 * b + 64],
                    [[0, 2], [-1, 68]], GT, 0.0,
                    base=1, channel_multiplier=1,
                )
            nc.gpsimd.affine_select(
                mm_sb[:], mm_sb[:], [[0, 2], [1, 68]], GE, 0.0, base=-1,
                channel_multiplier=0,
            )
            nc.gpsimd.affine_select(
                mm_sb[:], mm_sb[:], [[0, 2], [-1, 68]], GE, 0.0, base=65,
                channel_multiplier=0,
            )
            idents = idents_sb[:]
            ident = eye_sb[:]
            mmask_b = mm_sb[:]

            # ---- PSUM groups, one bank per (pair, tensor): FT_p is F
            # transposed; PT_p is opened by the F*diag(s) seed and closed by
            # that pair's last chunk-sum prefix matmul. F = row 0 of the
            # pair's first bulk tile.
            # PT is ONE bank with rows (pair, d): every par matmul then
            # covers BOTH pairs in a single LDW+MM (lhsT = the whole 128-col
            # par slot), halving PE work and leaving one close on the tail
            PT_ps = pp.tile([128, 128], FP, tag="PT", name="PT_ps")
            FT_ps = [
                pp.tile([64, 128], FP, tag=f"FT{p}", name=f"FT_ps{p}")
                for p in range(NPAIR)
            ]
            for p in range(NPAIR):
                nc.tensor.matmul(
                    FT_ps[p][:],
                    lhsT=kts[(p, 0)][:, 0, :],
                    rhs=ident,
                    start=True,
                    stop=True,
                )
                nc.tensor.matmul(
                    PT_ps[64 * p : 64 * p + 64, :],
                    lhsT=kts[(p, 0)][:, 0, :],
                    rhs=idents,
                    start=True,
                    stop=False,
                    skip_group_check=True,
                )

            # ---- per-sub-tile fold chains + per-sub-tile prefix matmuls.
            # Each (pair, sub-tile) folds independently down to one row (the
            # first fold targets the upper half so row 0 survives in tile 0),
            # writing its own partial-sum slot; the PE folds every partial
            # into the pair's scaled-prefix PSUM bank as it appears.
            pars = [
                parp.tile([128, NPAIR, DIM], FP, tag=f"par{s}", name=f"par{s}")
                for s in range(nsub)
            ]
            for s, rs in enumerate(sizes):
                for p in (1, 0):
                    t = kts[(p, s)]
                    e = nc.gpsimd if (p == 1 and s < GP_CHAINS) else nc.vector
                    h = rs // 2
                    e.tensor_add(t[:, h:rs, :], t[:, h:rs, :], t[:, 0:h, :])
                    lo, xr = h, h
                    while xr > 2:
                        hh = xr // 2
                        e.tensor_add(
                            t[:, lo : lo + hh, :],
                            t[:, lo : lo + hh, :],
                            t[:, lo + hh : lo + xr, :],
                        )
                        xr = hh
                    e.tensor_add(
                        pars[s][:, p, :], t[:, lo, :], t[:, lo + 1, :]
                    )
                nc.tensor.matmul(
                    PT_ps[:],
                    lhsT=pars[s][:].rearrange("q p d -> q (p d)"),
                    rhs=lmat_s,
                    start=False,
                    stop=s == nsub - 1,
                    skip_group_check=True,
                )

            # ---- sort projections (per pair), batched softmax (both pairs)
            PT_sb = [
                small.tile([64, 128], FP, tag=f"PTs{p}", name=f"PT_sb{p}")
                for p in range(NPAIR)
            ]
            FT_sb = [
                small.tile([64, 128], FP, tag=f"FTs{p}", name=f"FT_sb{p}")
                for p in range(NPAIR)
            ]
            for p in range(NPAIR):
                nc.scalar.copy(FT_sb[p][:], FT_ps[p][:])
                # engines can read a PSUM partition offset and write SBUF
                # partition 0, so both pairs' SKQ matmuls stay at base 0
                nc.scalar.copy(PT_sb[p][:], PT_ps[64 * p : 64 * p + 64, :])

            # SKQ rows: 0:40 sort-q blocks (b0 at 0:8, b1 at 32:40),
            #           64:104 sort-k blocks (b0 at 64:72, b1 at 96:104);
            # one PSUM bank per pair: each holds a long-open accumulation group
            # opened by the constant-term matmul (ready at kernel start) and
            # closed by the PT-part matmul (the only one on the critical tail)
            SQs = []
            RKs = []
            for p in range(NPAIR):
                sk_ps_t = pp.tile([104, 128], FP, tag=f"SKQ{p}")
                sk_ps = sk_ps_t[:]
                nc.tensor.matmul(
                    sk_ps,
                    lhsT=caux_sb[:, 256:360],
                    rhs=caux_sb[:, 128 * p : 128 * p + 128],
                    start=True,
                    stop=False,
                    skip_group_check=True,
                )
                nc.tensor.matmul(
                    sk_ps,
                    lhsT=c64_sb[:, 208 + 104 * p : 312 + 104 * p],
                    rhs=FT_sb[p][:],
                    start=False,
                    stop=False,
                    skip_group_check=True,
                )
                nc.tensor.matmul(
                    sk_ps,
                    lhsT=c64_sb[:, 104 * p : 104 * p + 104],
                    rhs=PT_sb[p][:],
                    start=False,
                    stop=True,
                    skip_group_check=True,
                )
                sq_sb = small.tile([40, 128], FP, tag=f"SQ{p}")
                nc.scalar.copy(sq_sb[:], sk_ps[0:40, :])
                rk_sb = small.tile([40, 128], FP, tag=f"RK{p}")
                nc.vector.tensor_copy(rk_sb[:], sk_ps[64:104, :])
                SQs.append(sq_sb)
                RKs.append(rk_sb)

            # R group: opened early by an identity-weighted matmul that seeds
            # the bank with the additive causal mask; the four sq.sk matmuls
            # then accumulate into their quadrants, so the masked logits sit
            # in PSUM with no extra elementwise pass
            # 68-wide per-pair blocks: col 0 pad, col 1 the pad-row's
            # constant zero logit (both from the mask seed), cols 2:66 the
            # sq.sk logits, 66:68 pad; width 68 keeps the partition-64
            # quadrant writes aligned to the PSUM zero-region window
            R_ps = pp.tile([128, 2 * 68], FP, tag="R")
            nc.tensor.matmul(
                R_ps[:],
                lhsT=ident,
                rhs=am_sb[:].rearrange("q p j -> q (p j)"),
                start=True,
                stop=False,
                skip_group_check=True,
            )
            for p in range(NPAIR):
                nc.tensor.matmul(
                    R_ps[0:64, 68 * p + 2 : 68 * p + 66],
                    lhsT=SQs[p][0:8, 0:64],
                    rhs=RKs[p][0:8, 0:64],
                    start=False,
                    stop=False,
                    skip_group_check=True,
                )
                nc.tensor.matmul(
                    R_ps[64:128, 68 * p + 2 : 68 * p + 66],
                    lhsT=SQs[p][32:40, 64:128],
                    rhs=RKs[p][32:40, 64:128],
                    start=False,
                    stop=p == NPAIR - 1,
                    skip_group_check=True,
                )

            # masked softmax over the 65 logits (cols 1:66 of each block;
            # pad cols give exp(NEG) = 0), both pairs batched: cols = (p, j)
            Rm = R_ps[:].rearrange("q (p j) -> q p j", p=2)
            mx = small.tile([128, 2], FP, tag="mx")
            nc.vector.reduce_max(mx[:], Rm, axis=X)
            negm = small.tile([128, 2], FP, tag="negm")
            nc.vector.tensor_scalar(
                negm[:], mx[:], 0.0, -1.0,
                op0=mybir.AluOpType.max, op1=MULT,
            )
            e_sb = small.tile([128, 2, 68], FP, tag="e")
            for p in range(NPAIR):
                nc.scalar.activation(
                    e_sb[:, p, :], R_ps[:, 68 * p : 68 * p + 68], Exp,
                    bias=negm[:, p : p + 1], scale=1.0,
                )
            s1 = small.tile([128, 2], FP, tag="s1")
            nc.vector.reduce_sum(s1[:], e_sb[:], axis=X)
            rin = small.tile([128, 2], FP, tag="rin")
            nc.vector.reciprocal(rin[:], s1[:])
            outt = small.tile([128, 2, 68], FP, tag="outt")
            for p in range(NPAIR):
                # outt = (e * 1/den) * tril-mask, fused
                nc.vector.scalar_tensor_tensor(
                    outt[:, p, :],
                    e_sb[:, p, :],
                    rin[:, p : p + 1],
                    mmask_b[:, p, :],
                    op0=MULT,
                    op1=MULT,
                )
            nc.sync.dma_start(
                rout.rearrange("(p b) i c -> (b i) p c", p=2),
                outt[:, :, 1:66],
            )

    nc.compile()
    return nc


def _get_program(t_seq=T, enable_asserts=False):
    key = (t_seq, enable_asserts)
    if key not in _PROG_CACHE:
        _PROG_CACHE[key] = _build_program(t_seq, enable_asserts=enable_asserts)
    return _PROG_CACHE[key]


def _host_constants(core, q_pos_emb, k_pos_emb, Wsq, Wsk, chunk=CHUNK):
    """Packed per-core constant tensors (two DMAs)."""
    f32 = np.float32
    j = np.arange(64, dtype=np.float64)
    s = (1.0 / (chunk * j + 1.0)).astype(f32)  # per-bucket cumavg scale

    tri = np.triu(np.ones((64, 64), f32), k=1)  # [c, j] = 1 iff c < j
    tri_s = tri * s[None, :]
    lmat_s = np.zeros((128, 128), f32)
    lmat_s[0:64, 0:64] = tri_s
    lmat_s[64:128, 64:128] = tri_s
    idents = np.zeros((128, 128), f32)
    idents[np.arange(128), np.arange(128)] = np.concatenate([s, s])
    ident = np.eye(128, dtype=f32)

    c128 = np.concatenate(
        [lmat_s, np.concatenate([s, s]).reshape(128, 1)], axis=1
    )

    wq_pt = np.zeros((2, 64, 104), f32)   # [pair][d][sq 0:40 | sk 64:104]
    wq_ft = np.zeros((2, 64, 104), f32)
    # c104 packed: only the 4 nonzero 8-row bands (SKQ rows 0:8, 32:40,
    # 64:72, 96:104) as 32 rows, plus the (32 -> 104) row-scatter matrix P
    c104p = np.zeros((2, 32, 128), f32)
    P = np.zeros((32, 104), f32)
    for r in range(32):
        P[r, 32 * (r // 8) + (r % 8)] = 1.0
    for p in range(NPAIR):
        for b in range(2):
            bh = core * BHC + 2 * p + b
            h = bh % HEADS
            r0 = 32 * b
            wq_pt[p, :, r0 : r0 + 8] = Wsq[0, h, 0:64, :]
            wq_pt[p, :, 64 + r0 : 64 + r0 + 8] = Wsk[0, h, 0:64, :]
            wq_ft[p, :, r0 : r0 + 8] = Wsq[0, h, 64:128, :]
            wq_ft[p, :, 64 + r0 : 64 + r0 + 8] = Wsk[0, h, 64:128, :]
            cq = q_pos_emb[0, h] @ Wsq[0, h, 128:192, :]  # (64, 8)
            ck = k_pos_emb[0, h] @ Wsk[0, h, 128:192, :]
            c104p[p, 8 * b : 8 * b + 8, 64 * b : 64 * b + 64] = cq.T
            c104p[p, 16 + 8 * b : 24 + 8 * b, 64 * b : 64 * b + 64] = ck.T

    c64 = np.concatenate([wq_pt[0], wq_pt[1], wq_ft[0], wq_ft[1]], axis=1)
    caux = np.concatenate([c104p[0], c104p[1], P], axis=1)
    assert c128.shape == (128, NCALL), c128.shape
    assert c64.shape == (64, NC64), c64.shape
    assert caux.shape == (32, NCAUX), caux.shape
    return {"cpack": c128, "c64": c64, "caux": caux}


def _run(k, q_pos_emb, k_pos_emb, Wsq, Wsk, trace=False, t_seq=T):
    nc = _get_program(t_seq)
    in_maps = []
    for core in range(NCORES):
        cm = _host_constants(
            core, q_pos_emb, k_pos_emb, Wsq, Wsk, chunk=t_seq // BUCKETS
        )
        cm["kin"] = np.ascontiguousarray(k[core * BHC : (core + 1) * BHC])
        in_maps.append(cm)
    res = bass_utils.run_bass_kernel_spmd(
        nc,
        in_maps,
        core_ids=list(range(NCORES)),
        trace=trace,
        **(TRACE_KWARGS if trace else {}),
    )
    global LAST_RESULTS
    LAST_RESULTS = res
    out = np.concatenate([r["rout"] for r in res.results], axis=0)
    return out, res


def kernel(**inputs):
    k = np.asarray(inputs["k"], np.float32)
    q_pos_emb = np.asarray(inputs["q_pos_emb"], np.float32)
    k_pos_emb = np.asarray(inputs["k_pos_emb"], np.float32)
    Wsq = np.asarray(inputs["Wsq"], np.float32)
    Wsk = np.asarray(inputs["Wsk"], np.float32)
    out, _ = _run(k, q_pos_emb, k_pos_emb, Wsq, Wsk, trace=TRACE)
    return out


# revision 27
# speedup vs baseline: 1.0092x; 1.0092x over previous
"""Trainium2 Bass kernel for CausalAttentionSortNet bucket-scoring.

Math (see reference): only `k` feeds the output. For each merged batch*head
slice, the cumulative-average of k is sampled at bucket starts (every 128th
row), which reduces to per-chunk sums + a strictly-triangular prefix matmul.
The rest is tiny per-bucket sort projections and a 64x65 masked softmax.

Sharding: data-parallel over the merged (batch*heads)=32 axis across 8 cores,
4 slices per core, processed as 2 pairs of 2 slices; a pair fills the
128-partition dim as partition=(slice_in_pair, chunk), free=(row, dim) so
every partition's k data is contiguous 32KB HBM runs (the single-queue bulk
stream saturates all 16 DMA engines at ~350 GB/s).

`q` (half of all input bytes) is never read by the reference computation, so
it is not even transferred to the device.

DMA-instruction budget: the hardware exposes ~12 DMA completion semaphores;
an instruction >=12 positions later reuses an earlier one's semaphore and
its issue blocks until that user completes, so the constants ship in two
early-completing DMAs and every bulk tile is uniform (a version with a slow
small-packet constant DMA in the reuse chain stalled the bulk queue 12us).
Chunk first-rows are not a separate DMA: they arrive inside each pair's
first bulk tile, whose in-place fold targets the tile's upper half so row 0
survives for the F-term matmuls.

Per-chunk reduction: each pair's rows stream as sub-tiles of
(16x7, 8, 4, 4) rows. Mid-stream, SBUF port contention caps DVE at roughly
1.7ns/elem and GpSimd at ~2.9ns/elem (vs 1.04/2.0 idle) and LARGER chains
degrade further (superlinear contention), so each sub-tile gets an
INDEPENDENT halving-fold chain (contiguous tensor_adds down to one row ->
its own partial-sum slot) and the chains are statically balanced across
both engines: GpSimd takes pair 1's first six chains, DVE everything else
including every chain near the tail. The PE (otherwise idle) folds every
partial into the scaled-prefix via one matmul per sub-tile against the
tril*scale constant, accumulating in that pair's PSUM bank, opened by the
F*diag(s) seed and closed by the last sub-tile's matmul. Small sub-tiles
stream last so the post-stream tail is two short fold chains plus the
epilogue (projections -> 64x65 masked softmax, batched over both pairs).
"""

from contextlib import ExitStack

import numpy as np

import concourse.bacc as bacc
import concourse.mybir as mybir
import concourse.tile as tile
from concourse import bass_utils

# Problem constants (hardcoded per contract; kernel.py must be self-contained).
B, HEADS, BUCKETS, DIM, DIM_SORT, T = 4, 8, 64, 64, 8, 8192
BH = B * HEADS            # 32 merged batch*head slices
NCORES = 8
BHC = BH // NCORES        # 4 slices per core
NPAIR = BHC // 2          # 2 pairs per core
CHUNK = T // BUCKETS      # 128 rows per bucket
NEG = -1.0e30             # softmax mask value (underflows exp to exactly 0)
FP = mybir.dt.float32

# packed-constant column offsets: cpack = [lmat_s | s2col], caux packs the
# nonzero rows of the cq/ck seed blocks plus their row-scatter matrix; the
# other structural constants (identity, diag(s), masks) are built on-chip
# by GpSimd affine_selects during its pre-stream idle window.
NC128 = 128 + 1
NC64 = 4 * 104
NCAUX = 2 * 128 + 104
NCALL = NC128

# pair-1 fold chains for sub-tiles [0, GP_CHAINS) run on GpSimd; all other
# chains (including every chain near the tail) on the faster DVE
GP_CHAINS = 6

TRACE = False  # set by test.py for profiling runs
TRACE_KWARGS = {}  # extra run_bass_kernel_spmd kwargs for profiling runs
LAST_RESULTS = None  # BassKernelResults of the most recent run

_PROG_CACHE = {}


def _cascade_sizes(chunk):
    # uniform mid-size tiles, small ones last: (16x7, 8, 4, 4) for chunk=128
    assert chunk == 128, "sub-tile schedule is tuned for chunk=128"
    sizes = [16] * 7 + [8, 4, 4]
    assert sum(sizes) == chunk, (sizes, chunk)
    return sizes


def _build_program(t_seq=T, enable_asserts=False, debug_taps=False):
    chunk = t_seq // BUCKETS
    sizes = _cascade_sizes(chunk)
    nsub = len(sizes)

    nc = bacc.Bacc(
        "TRN2",
        target_bir_lowering=False,
        debug=False,
        enable_asserts=enable_asserts,
        num_devices=NCORES,
    )

    def din(name, shape):
        return nc.dram_tensor(name, shape, FP, kind="ExternalInput").ap()

    kin = din("kin", (BHC, t_seq, DIM))
    # packed constants, three small DMAs:
    # cpack (128, 129)  [lmat_s | s-column]
    # c64   (64, 416)   [wqk_pt_p0 | wqk_pt_p1 | wqk_ft_p0 | wqk_ft_p1]
    # caux  (32, 360)   [c104 packed nonzero rows (256) | row-scatter P (104)]
    cpack = din("cpack", (128, NCALL))
    c64 = din("c64", (64, NC64))
    caux = din("caux", (32, NCAUX))
    rout = nc.dram_tensor(
        "rout", (BHC, BUCKETS, BUCKETS + 1), FP, kind="ExternalOutput"
    ).ap()

    X = mybir.AxisListType.X
    Exp = mybir.ActivationFunctionType.Exp
    MULT = mybir.AluOpType.mult

    with tile.TileContext(nc) as tc:
        with ExitStack() as ctx:
            singles = ctx.enter_context(tc.tile_pool(name="singles", bufs=1))
            kpools = [
                ctx.enter_context(tc.tile_pool(name=f"kpool{s}", bufs=2))
                for s in range(nsub)
            ]
            parp = ctx.enter_context(tc.tile_pool(name="parp", bufs=nsub))
            small = ctx.enter_context(tc.tile_pool(name="small", bufs=2))
            pp = ctx.enter_context(tc.tile_pool(name="pp", bufs=1, space="PSUM"))

            cp_sb = singles.tile([128, NCALL], FP, tag="cpack")
            nc.scalar.dma_start(cp_sb[:], cpack)
            c64_sb = singles.tile([64, NC64], FP, tag="c64")
            nc.scalar.dma_start(c64_sb[:], c64)
            caux_sb = singles.tile([32, NCAUX], FP, tag="caux")
            nc.scalar.dma_start(caux_sb[:], caux)

            # ---- bulk k sub-tile DMAs, single queue, pair 1 leading so
            # its GpSimd chains start first (contiguous rows*256B runs per
            # partition)
            ksrcs = [
                kin[2 * p : 2 * p + 2].rearrange("b (c r) d -> (b c) r d", r=chunk)
                for p in range(NPAIR)
            ]
            kts = {}
            r0 = 0
            for s, rs in enumerate(sizes):
                for p in (1, 0):
                    kt = kpools[s].tile(
                        [128, rs, DIM], FP, tag=f"kt{s}", name=f"kt{s}_{p}"
                    )
                    nc.sync.dma_start(kt[:], ksrcs[p][:, r0 : r0 + rs, :])
                    kts[(p, s)] = kt
                r0 += rs

            lmat_s = cp_sb[:, 0:128]
            s2col = cp_sb[:, 128:129]

            # on-chip structural constants, built by GpSimd (idle until the
            # first bulk tile lands ~11us in) + one Scalar scale op:
            #   eye      128x128 identity
            #   idents   diag(s) (cumavg scales)
            #   am68/mm68: additive causal mask / tril(-1) output mask in the
            #   68-wide-per-pair logit layout (col 0 pad, col 1+j logit j
            #   valid iff j <= row, cols 66:68 pad)
            # neuronxcc implements only is_ge / is_gt / not_equal for
            # affine_select, so every predicate is phrased with those
            GE, GT, NE = (
                mybir.AluOpType.is_ge,
                mybir.AluOpType.is_gt,
                mybir.AluOpType.not_equal,
            )
            eye_sb = singles.tile([128, 128], FP, tag="eye")
            nc.gpsimd.memset(eye_sb[:], 0.0)
            # where(c - i != 0, 0, fill=1) = identity
            nc.gpsimd.affine_select(
                eye_sb[:], eye_sb[:], [[1, 128]], NE, 1.0,
                base=0, channel_multiplier=-1,
            )
            idents_sb = singles.tile([128, 128], FP, tag="idents")
            nc.scalar.activation(
                idents_sb[:], eye_sb[:], mybir.ActivationFunctionType.Copy,
                scale=s2col,
            )
            am_sb = singles.tile([128, 2, 68], FP, tag="am68")
            nc.gpsimd.memset(am_sb[:], 0.0)
            for b in range(2):
                # keep 0 where row - pos + 1 >= 0 (valid logit), else NEG
                nc.gpsimd.affine_select(
                    am_sb[64 * b : 64 * b + 64], am_sb[64 * b : 64 * b + 64],
                    [[0, 2], [-1, 68]], GE, NEG,
                    base=1, channel_multiplier=1,
                )
            # pad columns: keep pos - 1 >= 0, keep 65 - pos >= 0, else NEG
            nc.gpsimd.affine_select(
                am_sb[:], am_sb[:], [[0, 2], [1, 68]], GE, NEG, base=-1,
                channel_multiplier=0,
            )
            nc.gpsimd.affine_select(
                am_sb[:], am_sb[:], [[0, 2], [-1, 68]], GE, NEG, base=65,
                channel_multiplier=0,
            )
            mm_sb = singles.tile([128, 2, 68], FP, tag="mm68")
            nc.gpsimd.memset(mm_sb[:], 1.0)
            for b in range(2):
                # keep 1 where row - pos + 1 > 0 (output tril(-1)), else 0
                nc.gpsimd.affine_select(
                    mm_sb[64 * b : 64 * b + 64], mm_sb[64 * b : 64 * b + 64],
                    [[0, 2], [-1, 68]], GT, 0.0,
                    base=1, channel_multiplier=1,
                )
            nc.gpsimd.affine_select(
                mm_sb[:], mm_sb[:], [[0, 2], [1, 68]], GE, 0.0, base=-1,
                channel_multiplier=0,
            )
            nc.gpsimd.affine_select(
                mm_sb[:], mm_sb[:], [[0, 2], [-1, 68]], GE, 0.0, base=65,
                channel_multiplier=0,
            )
            idents = idents_sb[:]
            ident = eye_sb[:]
            mmask_b = mm_sb[:]

            # ---- PSUM groups, one bank per (pair, tensor): FT_p is F
            # transposed; PT_p is opened by the F*diag(s) seed and closed by
            # that pair's last chunk-sum prefix matmul. F = row 0 of the
            # pair's first bulk tile.
            # PT is ONE bank with rows (pair, d): every par matmul then
            # covers BOTH pairs in a single LDW+MM (lhsT = the whole 128-col
            # par slot), halving PE work and leaving one close on the tail
            PT_ps = pp.tile([128, 128], FP, tag="PT", name="PT_ps")
            FT_ps = [
                pp.tile([64, 128], FP, tag=f"FT{p}", name=f"FT_ps{p}")
                for p in range(NPAIR)
            ]
            for p in range(NPAIR):
                nc.tensor.matmul(
                    FT_ps[p][:],
                    lhsT=kts[(p, 0)][:, 0, :],
                    rhs=ident,
                    start=True,
                    stop=True,
                )
                nc.tensor.matmul(
                    PT_ps[64 * p : 64 * p + 64, :],
                    lhsT=kts[(p, 0)][:, 0, :],
                    rhs=idents,
                    start=True,
                    stop=False,
                    skip_group_check=True,
                )

            # ---- per-sub-tile fold chains + per-sub-tile prefix matmuls.
            # Each (pair, sub-tile) folds independently down to one row (the
            # first fold targets the upper half so row 0 survives in tile 0),
            # writing its own partial-sum slot; the PE folds every partial
            # into the pair's scaled-prefix PSUM bank as it appears.
            pars = [
                parp.tile([128, NPAIR, DIM], FP, tag=f"par{s}", name=f"par{s}")
                for s in range(nsub)
            ]
            for s, rs in enumerate(sizes):
                for p in (1, 0):
                    t = kts[(p, s)]
                    e = nc.gpsimd if (p == 1 and s < GP_CHAINS) else nc.vector
                    h = rs // 2
                    e.tensor_add(t[:, h:rs, :], t[:, h:rs, :], t[:, 0:h, :])
                    lo, xr = h, h
                    while xr > 2:
                        hh = xr // 2
                        e.tensor_add(
                            t[:, lo : lo + hh, :],
                            t[:, lo : lo + hh, :],
                            t[:, lo + hh : lo + xr, :],
                        )
                        xr = hh
                    e.tensor_add(
                        pars[s][:, p, :], t[:, lo, :], t[:, lo + 1, :]
                    )
                nc.tensor.matmul(
                    PT_ps[:],
                    lhsT=pars[s][:].rearrange("q p d -> q (p d)"),
                    rhs=lmat_s,
                    start=False,
                    stop=s == nsub - 1,
                    skip_group_check=True,
                )

            # ---- sort projections (per pair), batched softmax (both pairs)
            PT_sb = [
                small.tile([64, 128], FP, tag=f"PTs{p}", name=f"PT_sb{p}")
                for p in range(NPAIR)
            ]
            FT_sb = [
                small.tile([64, 128], FP, tag=f"FTs{p}", name=f"FT_sb{p}")
                for p in range(NPAIR)
            ]
            for p in range(NPAIR):
                nc.scalar.copy(FT_sb[p][:], FT_ps[p][:])
                # engines can read a PSUM partition offset and write SBUF
                # partition 0, so both pairs' SKQ matmuls stay at base 0
                nc.scalar.copy(PT_sb[p][:], PT_ps[64 * p : 64 * p + 64, :])

            # SKQ rows: 0:40 sort-q blocks (b0 at 0:8, b1 at 32:40),
            #           64:104 sort-k blocks (b0 at 64:72, b1 at 96:104);
            # one PSUM bank per pair: each holds a long-open accumulation group
            # opened by the constant-term matmul (ready at kernel start) and
            # closed by the PT-part matmul (the only one on the critical tail)
            SQs = []
            RKs = []
            for p in range(NPAIR):
                sk_ps_t = pp.tile([104, 128], FP, tag=f"SKQ{p}")
                sk_ps = sk_ps_t[:]
                nc.tensor.matmul(
                    sk_ps,
                    lhsT=caux_sb[:, 256:360],
                    rhs=caux_sb[:, 128 * p : 128 * p + 128],
                    start=True,
                    stop=False,
                    skip_group_check=True,
                )
                nc.tensor.matmul(
                    sk_ps,
                    lhsT=c64_sb[:, 208 + 104 * p : 312 + 104 * p],
                    rhs=FT_sb[p][:],
                    start=False,
                    stop=False,
                    skip_group_check=True,
                )
                nc.tensor.matmul(
                    sk_ps,
                    lhsT=c64_sb[:, 104 * p : 104 * p + 104],
                    rhs=PT_sb[p][:],
                    start=False,
                    stop=True,
                    skip_group_check=True,
                )
                sq_sb = small.tile([40, 128], FP, tag=f"SQ{p}")
                nc.scalar.copy(sq_sb[:], sk_ps[0:40, :])
                rk_sb = small.tile([40, 128], FP, tag=f"RK{p}")
                nc.vector.tensor_copy(rk_sb[:], sk_ps[64:104, :])
                SQs.append(sq_sb)
                RKs.append(rk_sb)

            # R group: opened early by an identity-weighted matmul that seeds
            # the bank with the additive causal mask; the four sq.sk matmuls
            # then accumulate into their quadrants, so the masked logits sit
            # in PSUM with no extra elementwise pass
            # 68-wide per-pair blocks: col 0 pad, col 1 the pad-row's
            # constant zero logit (both from the mask seed), cols 2:66 the
            # sq.sk logits, 66:68 pad; width 68 keeps the partition-64
            # quadrant writes aligned to the PSUM zero-region window
            R_ps = pp.tile([128, 2 * 68], FP, tag="R")
            nc.tensor.matmul(
                R_ps[:],
                lhsT=ident,
                rhs=am_sb[:].rearrange("q p j -> q (p j)"),
                start=True,
                stop=False,
                skip_group_check=True,
            )
            for p in range(NPAIR):
                nc.tensor.matmul(
                    R_ps[0:64, 68 * p + 2 : 68 * p + 66],
                    lhsT=SQs[p][0:8, 0:64],
                    rhs=RKs[p][0:8, 0:64],
                    start=False,
                    stop=False,
                    skip_group_check=True,
                )
                nc.tensor.matmul(
                    R_ps[64:128, 68 * p + 2 : 68 * p + 66],
                    lhsT=SQs[p][32:40, 64:128],
                    rhs=RKs[p][32:40, 64:128],
                    start=False,
                    stop=p == NPAIR - 1,
                    skip_group_check=True,
                )

            # masked softmax over the 65 logits (cols 1:66 of each block;
            # pad cols give exp(NEG) = 0), both pairs batched: cols = (p, j)
            Rm = R_ps[:].rearrange("q (p j) -> q p j", p=2)
            mx = small.tile([128, 2], FP, tag="mx")
            nc.vector.reduce_max(mx[:], Rm, axis=X)
            negm = small.tile([128, 2], FP, tag="negm")
            nc.vector.tensor_scalar(
                negm[:], mx[:], 0.0, -1.0,
                op0=mybir.AluOpType.max, op1=MULT,
            )
            e_sb = small.tile([128, 2, 68], FP, tag="e")
            for p in range(NPAIR):
                nc.scalar.activation(
                    e_sb[:, p, :], R_ps[:, 68 * p : 68 * p + 68], Exp,
                    bias=negm[:, p : p + 1], scale=1.0,
                )
            s1 = small.tile([128, 2], FP, tag="s1")
            nc.vector.reduce_sum(s1[:], e_sb[:], axis=X)
            rin = small.tile([128, 2], FP, tag="rin")
            nc.vector.reciprocal(rin[:], s1[:])
            outt = small.tile([128, 2, 68], FP, tag="outt")
            for p in range(NPAIR):
                # outt = (e * 1/den) * tril-mask, fused
                nc.vector.scalar_tensor_tensor(
                    outt[:, p, :],
                    e_sb[:, p, :],
                    rin[:, p : p + 1],
                    mmask_b[:, p, :],
                    op0=MULT,
                    op1=MULT,
                )
            nc.sync.dma_start(
                rout.rearrange("(p b) i c -> (b i) p c", p=2),
                outt[:, :, 1:66],
            )

    nc.compile()
    return nc


def _get_program(t_seq=T, enable_asserts=False):
    key = (t_seq, enable_asserts)
    if key not in _PROG_CACHE:
        _PROG_CACHE[key] = _build_program(t_seq, enable_asserts=enable_asserts)
    return _PROG_CACHE[key]


def _host_constants(core, q_pos_emb, k_pos_emb, Wsq, Wsk, chunk=CHUNK):
    """Packed per-core constant tensors (two DMAs)."""
    f32 = np.float32
    j = np.arange(64, dtype=np.float64)
    s = (1.0 / (chunk * j + 1.0)).astype(f32)  # per-bucket cumavg scale

    tri = np.triu(np.ones((64, 64), f32), k=1)  # [c, j] = 1 iff c < j
    tri_s = tri * s[None, :]
    lmat_s = np.zeros((128, 128), f32)
    lmat_s[0:64, 0:64] = tri_s
    lmat_s[64:128, 64:128] = tri_s
    idents = np.zeros((128, 128), f32)
    idents[np.arange(128), np.arange(128)] = np.concatenate([s, s])
    ident = np.eye(128, dtype=f32)

    c128 = np.concatenate(
        [lmat_s, np.concatenate([s, s]).reshape(128, 1)], axis=1
    )

    wq_pt = np.zeros((2, 64, 104), f32)   # [pair][d][sq 0:40 | sk 64:104]
    wq_ft = np.zeros((2, 64, 104), f32)
    # c104 packed: only the 4 nonzero 8-row bands (SKQ rows 0:8, 32:40,
    # 64:72, 96:104) as 32 rows, plus the (32 -> 104) row-scatter matrix P
    c104p = np.zeros((2, 32, 128), f32)
    P = np.zeros((32, 104), f32)
    for r in range(32):
        P[r, 32 * (r // 8) + (r % 8)] = 1.0
    for p in range(NPAIR):
        for b in range(2):
            bh = core * BHC + 2 * p + b
            h = bh % HEADS
            r0 = 32 * b
            wq_pt[p, :, r0 : r0 + 8] = Wsq[0, h, 0:64, :]
            wq_pt[p, :, 64 + r0 : 64 + r0 + 8] = Wsk[0, h, 0:64, :]
            wq_ft[p, :, r0 : r0 + 8] = Wsq[0, h, 64:128, :]
            wq_ft[p, :, 64 + r0 : 64 + r0 + 8] = Wsk[0, h, 64:128, :]
            cq = q_pos_emb[0, h] @ Wsq[0, h, 128:192, :]  # (64, 8)
            ck = k_pos_emb[0, h] @ Wsk[0, h, 128:192, :]
            c104p[p, 8 * b : 8 * b + 8, 64 * b : 64 * b + 64] = cq.T
            c104p[p, 16 + 8 * b : 24 + 8 * b, 64 * b : 64 * b + 64] = ck.T

    c64 = np.concatenate([wq_pt[0], wq_pt[1], wq_ft[0], wq_ft[1]], axis=1)
    caux = np.concatenate([c104p[0], c104p[1], P], axis=1)
    assert c128.shape == (128, NCALL), c128.shape
    assert c64.shape == (64, NC64), c64.shape
    assert caux.shape == (32, NCAUX), caux.shape
    return {"cpack": c128, "c64": c64, "caux": caux}


def _run(k, q_pos_emb, k_pos_emb, Wsq, Wsk, trace=False, t_seq=T):
    nc = _get_program(t_seq)
    in_maps = []
    for core in range(NCORES):
        cm = _host_constants(
            core, q_pos_emb, k_pos_emb, Wsq, Wsk, chunk=t_seq // BUCKETS
        )
        cm["kin"] = np.ascontiguousarray(k[core * BHC : (core + 1) * BHC])
        in_maps.append(cm)
    res = bass_utils.run_bass_kernel_spmd(
        nc,
        in_maps,
        core_ids=list(range(NCORES)),
        trace=trace,
        **(TRACE_KWARGS if trace else {}),
    )
    global LAST_RESULTS
    LAST_RESULTS = res
    out = np.concatenate([r["rout"] for r in res.results], axis=0)
    return out, res


def kernel(**inputs):
    k = np.asarray(inputs["k"], np.float32)
    q_pos_emb = np.asarray(inputs["q_pos_emb"], np.float32)
    k_pos_emb = np.asarray(inputs["k_pos_emb"], np.float32)
    Wsq = np.asarray(inputs["Wsq"], np.float32)
    Wsk = np.asarray(inputs["Wsk"], np.float32)
    out, _ = _run(k, q_pos_emb, k_pos_emb, Wsq, Wsk, trace=TRACE)
    return out
